# revision 3
# baseline (speedup 1.0000x reference)
"""Trainium2 Bass kernel for the coverage-attention module (fp8 DoubleRow, v2).

Math (per batch b):
    enc_feat = encoder_outputs @ W_h.T                      [S, H]
    dec_fea  = s_t_hat @ W_s.T + b_s                        [H]
    e        = tanh(enc_feat + dec_fea + coverage[:,None]*W_c[:,0])
    scores   = e @ v[0]                                     [S]
    w        = exp(scores) * mask          (softmax+mask+renorm == w/sum(w))
    attn     = w / sum(w)
    c_t      = attn @ encoder_outputs                       [H]
    coverage_new = coverage + attn

Distribution: pure data-parallel over batch, 8 batches per NeuronCore,
weights replicated.  No collectives.

v2 design notes (vs the 431us v1):
  - startup: W_h (sync queue) and W_s (scalar queue) load in parallel as
    2-row-block pair DMAs; enc b0/b1 pairs queue right behind them.  No
    dec DRAM bounce: all 8 batches' fold operand tiles (fl/fr) are built
    at startup from dec8_sb / cov8 via tiny SBUF->SBUF DMAs on gpsimd.
  - SE=1: enc is NOT pre-scaled for fp8, so every bf16->fp8 cast is a
    plain tensor_copy (engine-flexible).
  - enc transposes: s-blocks {0,2,4,6} via the DMA xbar (sync queue,
    ~1.3us per s-block, single serial unit), {1,3,5,7} + batch 0 via PE
    bf16 transposes into PSUM (fp8 cast then reads PSUM).
  - PE p-state: idle gaps drop the PE from 1.2GHz (216ns per 512-col DR
    matmul) to 0.65GHz; it never reaches 2.4GHz on this part.  The
    schedule keeps the PE stream dense.
  - output DMAs are issued from gpsimd (SWDGE) to unload scalar.
"""

import numpy as np
import ml_dtypes

import concourse.bass as bass
import concourse.tile as tile
from concourse import bacc, mybir
from concourse.bass_utils import run_bass_kernel_spmd

N_CORES = 8
B, S, H = 64, 1024, 1024
BL = B // N_CORES  # batches per core

F32 = mybir.dt.float32
BF16 = mybir.dt.bfloat16
F8 = mybir.dt.float8e4
ALU = mybir.AluOpType
ACTF = mybir.ActivationFunctionType
DR = mybir.MatmulPerfMode.DoubleRow

SB = S // 128   # 8 s-blocks per batch
HB = H // 128   # 8 h-blocks
OCH = H // 512  # 2 o-chunks (PSUM bank width)

SW = 64.0       # W_h fp8 scale (enc unscaled: SE=1)
SFOLD = 8.0     # fold lhs-ones / rhs-dec scale (SFOLD^2 == SW)
INV = 1.0 / SW


def _build_kernel(tc, aps):
    nc = tc.nc
    enc, sth, mask, cov, wh, ws, bs, wc, v = (
        aps["encoder_outputs"], aps["s_t_hat"], aps["enc_padding_mask"],
        aps["coverage"], aps["W_h"], aps["W_s"], aps["b_s"], aps["W_c"], aps["v"],
    )
    ct_o, at_o, cn_o = aps["ct_out"], aps["attn_out"], aps["covnew_out"]

    id_dram = nc.inline_tensor(np.eye(128, dtype=ml_dtypes.bfloat16), name="id128")
    idf_dram = nc.inline_tensor(np.eye(128, dtype=np.float32), name="id128f")

    from contextlib import ExitStack
    ctx = ExitStack()
    with ctx:
        # ---------------- pools ----------------
        consts = ctx.enter_context(tc.tile_pool(name="consts", bufs=1))
        wpool = ctx.enter_context(tc.tile_pool(name="wpool", bufs=1))
        natf = ctx.enter_context(tc.tile_pool(name="natf", bufs=7))
        natps = [ctx.enter_context(tc.tile_pool(name=f"natp{i}", bufs=8))
                 for i in range(2)]
        encTp = ctx.enter_context(tc.tile_pool(name="encTp", bufs=5))
        enc8ps = [ctx.enter_context(tc.tile_pool(name=f"enc8p{i}", bufs=8))
                  for i in range(2)]
        ep = ctx.enter_context(tc.tile_pool(name="ep", bufs=4))
        vscrp = ctx.enter_context(tc.tile_pool(name="vscrp", bufs=3))
        smp = ctx.enter_context(tc.tile_pool(name="smp", bufs=2))

        # ---------------- constants ----------------
        id_bf = consts.tile([128, 128], BF16, tag="id")
        nc.gpsimd.dma_start(id_bf[:], id_dram.ap())
        id_f = consts.tile([128, 128], F32, tag="idf")
        nc.gpsimd.dma_start(id_f[:], idf_dram.ap())
        ones_1x128_bf = consts.tile([1, 128], BF16, tag="o1x128b")
        nc.vector.memset(ones_1x128_bf[:], 1.0)
        ones_1x128_f = consts.tile([1, 128], F32, tag="o1x128f")
        nc.vector.memset(ones_1x128_f[:], 1.0)
        ones_1x8_bf = consts.tile([1, 8], BF16, tag="o1x8b")
        nc.vector.memset(ones_1x8_bf[:], 1.0)
        ones_col_f = consts.tile([128, 1], F32, tag="ocolf")
        nc.vector.memset(ones_col_f[:], 1.0)

        # ---- startup loads ----
        # W_h pairs on sync, W_s pairs on scalar (parallel); enc b0/b1
        # pairs are emitted after so they queue behind the weights.
        wh_f = [natf.tile([128, 2, H], F32, tag="natf", name=f"whf{j}")
                for j in range(4)]
        for j in range(4):
            nc.sync.dma_start(
                wh_f[j][:],
                wh[2 * j * 128:(2 * j + 2) * 128, :].rearrange(
                    "(a p) h -> p a h", p=128))
        ws_f = [natf.tile([128, 2, H], F32, tag="natf", name=f"wsf{j}")
                for j in range(4)]
        for j in range(4):
            nc.scalar.dma_start(
                ws_f[j][:],
                ws[2 * j * 128:(2 * j + 2) * 128, :].rearrange(
                    "(a p) h -> p a h", p=128))

        # tiny rows via SWDGE (gpsimd queue is otherwise idle at t=0)
        bs_row = consts.tile([1, H], BF16, tag="bsrow")
        nc.gpsimd.dma_start(bs_row[:], bs[:])
        v_row = consts.tile([1, H], BF16, tag="vrow")
        nc.gpsimd.dma_start(v_row[:], v[:, :])

        # ---------------- psum pools (shared startup + main loop) -------
        psum_e = ctx.enter_context(tc.tile_pool(name="psum_e", bufs=4,
                                                space="PSUM"))
        psum_tr = ctx.enter_context(tc.tile_pool(name="psum_tr", bufs=1,
                                                 space="PSUM"))
        psum_ct = ctx.enter_context(tc.tile_pool(name="psum_ct", bufs=2,
                                                 space="PSUM"))
        psum_z = ctx.enter_context(tc.tile_pool(name="psum_z", bufs=1,
                                                space="PSUM"))

        # ---------------- enc ingest pipeline ----------------
        enc_f32 = [[None] * 4 for _ in range(BL)]    # pair tiles
        enc_nat = [[None] * SB for _ in range(BL)]
        encbf_t = [[None] * SB for _ in range(BL)]
        enc8_t = [[None] * SB for _ in range(BL)]

        def load_pair(b, j):
            """One DMA for s-blocks 2j, 2j+1 of batch b."""
            t = natf.tile([128, 2, H], F32, tag="natf", name=f"ef{b}_{j}")
            enc_f32[b][j] = t
            eng = nc.sync if j % 2 == 0 else nc.scalar
            eng.dma_start(t[:], enc[b, 2 * j * 128:(2 * j + 2) * 128, :]
                          .rearrange("(a p) h -> p a h", p=128))

        def cast_nat(b, sb):
            nat = natps[b % 2].tile([128, H], BF16, tag="nat",
                                    name=f"en{b}_{sb}")
            enc_nat[b][sb] = nat
            nc.vector.tensor_copy(nat[:], enc_f32[b][sb // 2][:, sb % 2, :])

        def xbar_sb(b, sb, force_pe=False):
            """Even s-blocks via the (single, serial) xbar on the sync
            queue; odd s-blocks via PE bf16 transposes into PSUM (cheap:
            LDW pipelines at ~70ns/tile in-stream).  Batch 0 goes all-PE
            so it doesn't queue behind startup loads on sync."""
            if sb % 2 == 0 and not force_pe:
                encT = encTp.tile([128, HB, 128], BF16, tag="encbf",
                                  name=f"eT{b}_{sb}")
                encbf_t[b][sb] = encT
                nc.sync.dma_start(encT[:], enc_nat[b][sb][:], transpose=True)
            else:
                ptr = psum_tr.tile([128, HB, 128], BF16, tag="etr",
                                   name=f"eptr{b}_{sb}")
                encbf_t[b][sb] = ptr
                for hb in range(HB):
                    nc.tensor.transpose(ptr[:, hb, :],
                                        enc_nat[b][sb][:, hb * 128:(hb + 1) * 128],
                                        id_bf[:])

        def cast8_sb(b, sb):
            e8 = enc8ps[b % 2].tile([128, HB, 128], F8, tag="enc8",
                                    name=f"e8{b}_{sb}")
            enc8_t[b][sb] = e8
            if b == 0 and sb % 2 == 0:
                nc.scalar.copy(e8[:], encbf_t[b][sb][:])
            elif b == 0 or sb in (0, 2, 3, 5, 7):
                nc.vector.tensor_copy(e8[:], encbf_t[b][sb][:])
            elif sb in (1, 4):
                nc.scalar.copy(e8[:], encbf_t[b][sb][:])
            else:
                nc.gpsimd.tensor_copy(e8[:], encbf_t[b][sb][:])

        # b0 + b1 enc loads: behind the weights on both queues
        for j in range(4):
            load_pair(0, j)
        for j in range(4):
            load_pair(1, j)

        # ---- W processing ----
        startp_cm = tc.tile_pool(name="startp", bufs=1)
        startp = startp_cm.__enter__()
        wc_f32 = startp.tile([1, H], F32, tag="wcf32")
        nc.gpsimd.dma_start(wc_f32[:], wc[:, :])
        m_rows = startp.tile([BL, S], F32, tag="mrows")
        nc.gpsimd.dma_start(m_rows[:], mask[:, :])
        c_rows = startp.tile([BL, S], F32, tag="crows")
        nc.gpsimd.dma_start(c_rows[:], cov[:, :])

        # W_h: f32 -> bf16*SW rows (DVE), PE transpose, fp8 copy-out
        wh8T = wpool.tile([128, HB, H], F8, tag="wh8T")   # [h_p, hb, o]
        for j in range(4):
            whb_j = startp.tile([128, 2, H], BF16, tag="whb", bufs=2,
                                name=f"whb{j}")
            if j % 2 == 0:
                nc.vector.tensor_scalar_mul(whb_j[:], wh_f[j][:], SW)
            else:
                nc.scalar.activation(whb_j[:], wh_f[j][:], ACTF.Copy,
                                     scale=SW)
            for a in range(2):
                ob = 2 * j + a
                ptr = psum_e.tile([128, HB, 128], BF16, tag="pe")
                for hb in range(HB):
                    nc.tensor.transpose(ptr[:, hb, :],
                                        whb_j[:, a, hb * 128:(hb + 1) * 128],
                                        id_bf[:])
                if ob % 2 == 0:
                    nc.vector.tensor_copy(
                        wh8T[:, :, ob * 128:(ob + 1) * 128], ptr[:])
                else:
                    nc.scalar.copy(wh8T[:, :, ob * 128:(ob + 1) * 128], ptr[:])

        # b0 enc chain (its pairs land during W processing): all-PE
        # transposes + DVE casts.
        for sb in range(SB):
            cast_nat(0, sb)
            xbar_sb(0, sb, force_pe=True)
            cast8_sb(0, sb)

        # W_s: f32 -> bf16 rows (scalar), PE transpose, bf16 copy-out
        # (scalar) -- keeps the DVE free for the b0 chain.
        with tc.tile_pool(name="wsTp", bufs=1) as wsTp:
            wsT = wsTp.tile([128, HB, H], BF16, tag="wsT")
            for j in range(4):
                wsb = startp.tile([128, 2, H], BF16, tag="wsb", bufs=2,
                                  name=f"wsb{j}")
                nc.scalar.copy(wsb[:], ws_f[j][:])
                for a in range(2):
                    ob = 2 * j + a
                    pool = psum_tr if ob % 2 == 0 else psum_e
                    ptr = pool.tile([128, HB, 128], BF16,
                                    tag="etr" if ob % 2 == 0 else "pe")
                    for hb in range(HB):
                        nc.tensor.transpose(ptr[:, hb, :],
                                            wsb[:, a, hb * 128:(hb + 1) * 128],
                                            id_bf[:])
                    nc.scalar.copy(wsT[:, :, ob * 128:(ob + 1) * 128],
                                   ptr[:])

            # s_t_hat^T (SWDGE casts f32->bf16 during the DMA)
            s_bf = consts.tile([BL, H], BF16, tag="sbf")
            nc.gpsimd.dma_start(s_bf[:], sth[:, :])
            sT = consts.tile([128, HB, BL], BF16, tag="sT")
            for hb in range(HB):
                ptr = psum_ct.tile([128, BL], BF16, tag="ct")
                nc.tensor.transpose(ptr[:], s_bf[:, hb * 128:(hb + 1) * 128],
                                    id_bf[0:BL, 0:BL])
                nc.scalar.copy(sT[:, hb, :], ptr[:])

            # dec_fea[b, o] = s_t_hat @ W_s.T + b_s  (PSUM partition = b)
            dec8_sb = consts.tile([BL, H], F8, tag="dec8sb")
            for och in range(OCH):
                osl = slice(och * 512, (och + 1) * 512)
                dec_ps = psum_e.tile([BL, 512], F32, tag="pe",
                                     name=f"decps{och}")
                for hb in range(HB):
                    nc.tensor.matmul(
                        dec_ps[:], sT[:, hb, :], wsT[:, hb, osl],
                        start=(hb == 0), stop=False)
                nc.tensor.matmul(
                    dec_ps[:], ones_1x8_bf[:], bs_row[:, osl],
                    start=False, stop=True)
                nc.scalar.activation(dec8_sb[:, osl], dec_ps[:], ACTF.Copy,
                                     scale=SFOLD)

            # v broadcast to all 128 partitions (via ones outer-product)
            v_bcast = consts.tile([128, H], BF16, tag="vbc")
            for och in range(OCH):
                osl = slice(och * 512, (och + 1) * 512)
                vb_ps = psum_e.tile([128, 512], F32, tag="pe",
                                    name=f"vbps{och}")
                nc.tensor.matmul(vb_ps[:], ones_1x128_bf[:], v_row[:, osl],
                                 start=True, stop=True)
                nc.scalar.copy(v_bcast[:, osl], vb_ps[:])

        # mask/cov -> column layout [s%128, b, s//128] via PE transposes
        mask_col = consts.tile([128, BL, SB], F32, tag="mcol")
        cov_col = consts.tile([128, BL, SB], F32, tag="ccol")
        for j in range(SB):
            jsl = slice(j * 128, (j + 1) * 128)
            mt = psum_ct.tile([128, BL], F32, tag="ct")
            nc.tensor.transpose(mt[:], m_rows[:, jsl], id_f[0:BL, 0:BL])
            nc.scalar.copy(mask_col[:, :, j], mt[:])
            ct_ = psum_ct.tile([128, BL], F32, tag="ct")
            nc.tensor.transpose(ct_[:], c_rows[:, jsl], id_f[0:BL, 0:BL])
            nc.scalar.copy(cov_col[:, :, j], ct_[:])

        # ---- fold operand tiles for all 8 batches (no DRAM bounce) ----
        # Per batch b (partitions 0-1):
        #   lhsT fl[b]: [p0,sub0]=SFOLD const, [p1,sub0]=cov8_b; sub1=0
        #   rhs  fr[b]: [p0,sub0]=dec8_b*SFOLD, [p1,sub0]=wc8*SW; sub1=0
        # so the DR matmul adds SFOLD^2*dec + cov*wc*SW = SW*(dec + cov*wc).
        cov8 = consts.tile([BL, S], F8, tag="cov8")
        nc.gpsimd.tensor_copy(cov8[:], c_rows[:])
        wc8_row = consts.tile([1, H], F8, tag="wc8row")
        nc.scalar.activation(wc8_row[:], wc_f32[:], ACTF.Copy, scale=SW)
        # 2-parity fold tiles, constants set once; the per-batch cov/dec
        # rows are refreshed from the persistent cov8/dec8_sb SBUF tiles
        # by tiny gpsimd DMAs one batch ahead (build_fold).
        fl_par = [consts.tile([2, 2, S], F8, tag=f"flp{i}", name=f"flp{i}")
                  for i in range(2)]
        fr_par = [consts.tile([2, 2, H], F8, tag=f"frp{i}", name=f"frp{i}")
                  for i in range(2)]
        for i in range(2):
            nc.gpsimd.memset(fl_par[i][:], 0.0)
            nc.gpsimd.memset(fl_par[i][0:1, 0:1, :], SFOLD)
            nc.gpsimd.memset(fr_par[i][:], 0.0)
            nc.gpsimd.dma_start(fr_par[i][1:2, 0:1, :], wc8_row[:, :])

        def build_fold(b):
            nc.gpsimd.dma_start(fl_par[b % 2][1:2, 0:1, :], cov8[b:b + 1, :])
            nc.gpsimd.dma_start(fr_par[b % 2][0:1, 0:1, :],
                                dec8_sb[b:b + 1, :])

        build_fold(0)
        fl_all = [fl_par[b % 2] for b in range(BL)]
        fr_all = [fr_par[b % 2] for b in range(BL)]
        startp_cm.__exit__(None, None, None)

        # ---------------- main loop ----------------
        def finish_batch(b, wm, ct_ps):
            """Z, 1/Z, attn/covnew/ct outputs for batch b.  Emitted inside
            batch b+1's stream so the PE never waits on the DVE chain."""
            rowsum = smp.tile([128, 1], F32, tag="rowsum")
            nc.vector.tensor_reduce(rowsum[:], wm[:], mybir.AxisListType.X,
                                    ALU.add)
            zps = psum_z.tile([1, 1], F32, tag="z")
            nc.tensor.matmul(zps[:], ones_col_f[:], rowsum[:], start=True,
                             stop=True)
            z_sb = smp.tile([1, 1], F32, tag="zsb")
            nc.vector.tensor_copy(z_sb[:], zps[:])
            zb_ps = psum_z.tile([128, 1], F32, tag="z")
            nc.tensor.matmul(zb_ps[:], ones_1x128_f[:], z_sb[:], start=True,
                             stop=True)
            zb = smp.tile([128, 1], F32, tag="zb")
            nc.vector.tensor_copy(zb[:], zb_ps[:])
            rz = smp.tile([128, 1], F32, tag="rz")
            nc.vector.reciprocal(rz[:], zb[:])

            attn_c = smp.tile([128, SB], F32, tag="attnc")
            nc.vector.tensor_scalar_mul(attn_c[:], wm[:], rz[:, 0:1])
            covn_c = smp.tile([128, SB], F32, tag="covnc")
            nc.vector.tensor_tensor(covn_c[:], attn_c[:], cov_col[:, b, :],
                                    ALU.add)
            # transpose [s128, j] -> [j, s128] so the output DMA is
            # contiguous (512B runs instead of 4B scatter packets)
            atp = psum_z.tile([BL, 128], F32, tag="z")
            nc.tensor.transpose(atp[:], attn_c[:], id_f[:])
            at_row = smp.tile([BL, 128], F32, tag="atrow")
            nc.scalar.copy(at_row[:], atp[:])
            nc.gpsimd.dma_start(at_o[b, :].rearrange("(j p) -> j p", p=128),
                                at_row[:])
            cvp = psum_z.tile([BL, 128], F32, tag="z")
            nc.tensor.transpose(cvp[:], covn_c[:], id_f[:])
            cv_row = smp.tile([BL, 128], F32, tag="cvrow")
            nc.scalar.copy(cv_row[:], cvp[:])
            nc.gpsimd.dma_start(cn_o[b, :].rearrange("(j p) -> j p", p=128),
                                cv_row[:])

            ct_sb = smp.tile([1, H], F32, tag="ctsb")
            for och in range(OCH):
                nc.vector.tensor_scalar_mul(
                    ct_sb[:, och * 512:(och + 1) * 512], ct_ps[och][:],
                    rz[0:1, 0:1])
            nc.gpsimd.dma_start(ct_o[b, :], ct_sb[:])

        prev_fin = None
        for b in range(BL):
            if b + 1 < BL:
                build_fold(b + 1)
            sc = smp.tile([128, SB], F32, tag="scores")
            we = smp.tile([128, SB], F32, tag="we")
            wm = smp.tile([128, SB], F32, tag="wm")
            w_bf = smp.tile([128, SB], BF16, tag="wbf")
            ct_ps = [psum_ct.tile([1, 512], F32, tag="ct", name=f"ctps{och}")
                     for och in range(OCH)]

            def ct_mm(sb, b=b, w_bf=w_bf, ct_ps=ct_ps):
                for och in range(OCH):
                    nc.tensor.matmul(
                        ct_ps[och][:], w_bf[:, sb:sb + 1],
                        enc_nat[b][sb][:, och * 512:(och + 1) * 512],
                        start=(sb == 0), stop=(sb == SB - 1))

            for sb in range(SB):
                # pipelined ingest: load b+2, cast b+1, xbar b+1 (lag 1),
                # fp8 b+1 (lag 2)
                if sb % 2 == 1 and b + 2 < BL:
                    load_pair(b + 2, sb // 2)
                if b + 1 < BL:
                    cast_nat(b + 1, sb)
                    if sb >= 1:
                        xbar_sb(b + 1, sb - 1)
                    if sb >= 2:
                        cast8_sb(b + 1, sb - 2)
                ssl = slice(sb * 128, (sb + 1) * 128)
                spart = smp.tile([128, OCH], F32, tag="spart")
                for och in range(OCH):
                    osl = slice(och * 512, (och + 1) * 512)
                    pe = psum_e.tile([128, 512], F32, tag="pe")
                    for k in range(HB // 2):
                        nc.tensor.matmul(pe[:],
                                         enc8_t[b][sb][:, 2 * k:2 * k + 2, :],
                                         wh8T[:, 2 * k:2 * k + 2, osl],
                                         start=(k == 0), stop=False,
                                         perf_mode=DR)
                    nc.tensor.matmul(pe[:], fl_all[b][:, :, ssl],
                                     fr_all[b][:, :, osl],
                                     start=False, stop=True, perf_mode=DR)
                    e_bf = ep.tile([128, 512], BF16, tag="e")
                    nc.scalar.activation(e_bf[:], pe[:], ACTF.Tanh, scale=INV)
                    vscr = vscrp.tile([128, 512], BF16, tag="vscr")
                    nc.vector.scalar_tensor_tensor(
                        out=vscr[:], in0=e_bf[:], scalar=1.0,
                        in1=v_bcast[:, osl], op0=ALU.mult, op1=ALU.mult,
                        accum_out=spart[:, och:och + 1])
                nc.vector.tensor_tensor(sc[:, sb:sb + 1], spart[:, 0:1],
                                        spart[:, 1:2], ALU.add)
                nc.scalar.activation(we[:, sb:sb + 1], sc[:, sb:sb + 1],
                                     ACTF.Exp)
                nc.vector.tensor_tensor(wm[:, sb:sb + 1], we[:, sb:sb + 1],
                                        mask_col[:, b, sb:sb + 1], ALU.mult)
                nc.vector.tensor_copy(w_bf[:, sb:sb + 1], wm[:, sb:sb + 1])
                if sb > 0:
                    ct_mm(sb - 1)
                if sb == 0 and prev_fin is not None:
                    prev_fin()
            if b + 1 < BL:
                xbar_sb(b + 1, SB - 1)
                cast8_sb(b + 1, SB - 2)
                cast8_sb(b + 1, SB - 1)
            ct_mm(SB - 1)
            prev_fin = (lambda b=b, wm=wm, ct_ps=ct_ps:
                        finish_batch(b, wm, ct_ps))
        prev_fin()


def build():
    nc = bacc.Bacc("TRN2", target_bir_lowering=False, debug=False,
                   num_devices=N_CORES)
    aps = {}
    aps["encoder_outputs"] = nc.dram_tensor(
        "encoder_outputs", [BL, S, H], F32, kind="ExternalInput").ap()
    aps["s_t_hat"] = nc.dram_tensor("s_t_hat", [BL, H], F32, kind="ExternalInput").ap()
    aps["enc_padding_mask"] = nc.dram_tensor(
        "enc_padding_mask", [BL, S], F32, kind="ExternalInput").ap()
    aps["coverage"] = nc.dram_tensor("coverage", [BL, S], F32, kind="ExternalInput").ap()
    aps["W_h"] = nc.dram_tensor("W_h", [H, H], F32, kind="ExternalInput").ap()
    aps["W_s"] = nc.dram_tensor("W_s", [H, H], F32, kind="ExternalInput").ap()
    aps["b_s"] = nc.dram_tensor("b_s", [H], F32, kind="ExternalInput").ap()
    aps["W_c"] = nc.dram_tensor("W_c", [H, 1], F32, kind="ExternalInput").ap()
    aps["v"] = nc.dram_tensor("v", [1, H], F32, kind="ExternalInput").ap()
    aps["ct_out"] = nc.dram_tensor("ct_out", [BL, H], F32, kind="ExternalOutput").ap()
    aps["attn_out"] = nc.dram_tensor("attn_out", [BL, S], F32, kind="ExternalOutput").ap()
    aps["covnew_out"] = nc.dram_tensor("covnew_out", [BL, S], F32, kind="ExternalOutput").ap()

    with tile.TileContext(nc) as tc:
        _build_kernel(tc, aps)
    nc.compile()
    return nc


_NC_CACHE = {}


def _get_nc():
    if "nc" not in _NC_CACHE:
        _NC_CACHE["nc"] = build()
    return _NC_CACHE["nc"]


def kernel(s_t_hat, encoder_outputs, enc_padding_mask, coverage,
           W_h, W_s, b_s, W_c, v, _trace=False, _tmpdir=None):
    f = lambda x: np.ascontiguousarray(np.asarray(x), dtype=np.float32)
    s_t_hat, encoder_outputs = f(s_t_hat), f(encoder_outputs)
    enc_padding_mask, coverage = f(enc_padding_mask), f(coverage)
    W_h, W_s, b_s, W_c, v = f(W_h), f(W_s), f(b_s), f(W_c), f(v)

    nc = _get_nc()
    in_maps = []
    for i in range(N_CORES):
        sl = slice(i * BL, (i + 1) * BL)
        in_maps.append({
            "encoder_outputs": encoder_outputs[sl],
            "s_t_hat": s_t_hat[sl],
            "enc_padding_mask": enc_padding_mask[sl],
            "coverage": coverage[sl],
            "W_h": W_h, "W_s": W_s, "b_s": b_s, "W_c": W_c, "v": v,
        })
    res = run_bass_kernel_spmd(nc, in_maps, core_ids=list(range(N_CORES)),
                               trace=_trace, tmpdir=_tmpdir)
    ct = np.concatenate([res.results[i]["ct_out"] for i in range(N_CORES)], axis=0)
    at = np.concatenate([res.results[i]["attn_out"] for i in range(N_CORES)], axis=0)
    cn = np.concatenate([res.results[i]["covnew_out"] for i in range(N_CORES)], axis=0)
    kernel._last_results = res
    return ct, at, cn
